# revision 28
# baseline (speedup 1.0000x reference)
"""ConcatRelationModule Bass kernel for 8 trn2 NeuronCores — v12.

Per edge e in [0, 16383):
    x      = concat(inputs[heads[e], 0, :], inputs[e + 1, 1, :])     # [512]
    h      = tanh(concat(x @ W_FOH, x @ W_FOM) + b1)                 # [1024]
    h2     = tanh(h @ W2 + b2)                                       # [256]
    out[e] = h2 @ W3 + b3                                            # [E, 64]

v12 core idea: the gather (gpsimd software descriptor generation,
~1.1us per 128 rows) is the pacing item early on, and any PE idle
>~3.4us re-throttles the HAM clock gate to half rate.  So L1 is split
into two passes per group of 256 edges:
  pass A: the modifier half (kc 2,3) — fed by a host-pretransposed
          direct DMA, no gather dependency
  pass B: the head half (kc 0,1) — closes the accumulation once the
          gather+transpose has landed
Eight hc accumulations stay open across the passes by packing hc pairs
into single PSUM banks ([P, 2, 256] f32 = one 2KB bank), so a group
needs 4 banks.  L2/L3 lag L1 by one group.  DMA deliveries are
priority-ordered: hT+gathers on gpsimd only; xm/bp/ident early on
scalar; w1 then w2 then cold tensors on sync.  Warm-up matmuls gated
only on a small DVE memset keep the PE (and HAM) busy from ~7us.
Output is stored bf16 and converted (+bias) on host.
"""

import os

import numpy as np
import ml_dtypes

import concourse.bass as bass
import concourse.bacc as bacc
import concourse.mybir as mybir
import concourse.tile as tile
from concourse.bass import IndirectOffsetOnAxis
from concourse.bass_utils import run_bass_kernel_spmd

N_TOKENS = 16384
LD = 256
HID = 512
HID2 = 256
NREL = 64
NCORES = 8
E = N_TOKENS - 1
EPC = N_TOKENS // NCORES  # 2048
P = 128
SUB = EPC // P            # 16
N_WARMUP = 12

# variable group sizes: small first groups (fast pipeline ramp: less
# data needed before the first matmuls / first gather) and small last
# groups (short drain tail)
GSIZES = [128, 128, 256, 256, 256, 256, 256, 256, 128, 128]
GROUPS = []
_o = 0
for _s in GSIZES:
    GROUPS.append((_o, _s))
    _o += _s
assert _o == EPC
NG = len(GROUPS)
GMAX = max(GSIZES)

LAST_RESULTS = None
_CACHE = {}


def _build():
    bf16 = mybir.dt.bfloat16
    f32 = mybir.dt.float32

    nc = bacc.Bacc()
    fwd = nc.declare_dram_parameter("fwd", [N_TOKENS, LD], bf16, isOutput=False)
    bwdT = nc.declare_dram_parameter("bwdT", [P, 2, EPC], bf16, isOutput=False)
    headsT = nc.declare_dram_parameter(
        "headsT", [P, SUB], mybir.dt.int32, isOutput=False)
    w1 = nc.declare_dram_parameter("w1", [2 * LD, 2 * HID], bf16, isOutput=False)
    w2p = nc.declare_dram_parameter("w2p", [P, 2, 8, HID2 // 2], bf16,
                                    isOutput=False)
    w3p = nc.declare_dram_parameter("w3p", [P, 2, NREL], bf16, isOutput=False)
    bpack = nc.declare_dram_parameter("bpack", [P, 10], f32, isOutput=False)
    identw = nc.declare_dram_parameter("identw", [P, P], bf16, isOutput=False)
    outT = nc.declare_dram_parameter("outT", [NREL, EPC], bf16, isOutput=True)

    Tanh = mybir.ActivationFunctionType.Tanh

    with tile.TileContext(nc) as tc:
        with (
            tc.tile_pool(name="const", bufs=1) as const_pool,
            tc.tile_pool(name="xh", bufs=NG) as xh_pool,
            tc.tile_pool(name="xm", bufs=NG) as xm_pool,
            tc.tile_pool(name="xT", bufs=4) as xT_pool,
            tc.tile_pool(name="h1", bufs=16) as h1_pool,
            tc.tile_pool(name="h2", bufs=4) as h2_pool,
            tc.tile_pool(name="outs", bufs=3) as out_pool,
            # PSUM: ph 4 banks (one open hc accumulation each), pt 2, pj 2.
            # NOTE: start_tensor_calc=True clears has_written for the WHOLE
            # bank, so only one open accumulation region may live per bank.
            tc.tile_pool(name="pt", bufs=2, space="PSUM") as pt_pool,
            tc.tile_pool(name="ph", bufs=4, space="PSUM") as ph_pool,
            tc.tile_pool(name="pj", bufs=2, space="PSUM") as pj_pool,
        ):
            # --- prologue (per-engine emission order == queue order) ---
            # DVE: warm-up scratch; first 128 cols land fast so the PE
            # warm-ups can start ~0.6us earlier
            warm_sb = const_pool.tile([P, 512], bf16)
            nc.vector.memset(warm_sb[:, 0:P], 0)
            nc.vector.memset(warm_sb[:, P:512], 0)

            # gpsimd: headsT (first two columns split out: gather(0) and
            # gather(1) gate only on the tiny first transfer), then gathers
            hT_sb = const_pool.tile([P, SUB], mybir.dt.int32)
            nc.gpsimd.dma_start(hT_sb[:, 0:2], headsT[:, 0:2])
            nc.gpsimd.dma_start(hT_sb[:, 2:SUB], headsT[:, 2:SUB])

            # PE warm-ups keep the HAM clock gate at K=8/8 (any >3.4us PE
            # idle re-throttles to half rate).  A first block runs during
            # the DMA prologue; more are sprinkled between the early
            # pipeline stages as dependency-free gap fillers.
            wps = pt_pool.tile([P, 512], f32, tag="pt", name="warmup")

            def emit_warm(n, free=512):
                for _ in range(n):
                    nc.tensor.matmul(
                        out=wps[:, 0:free], lhsT=warm_sb[:, 0:P],
                        rhs=warm_sb[:, 0:free], start=True, stop=True,
                    )

            emit_warm(4, free=P)
            emit_warm(7)

            xg_tiles = [None] * NG
            xm_tiles = [None] * NG

            def load_xm(gi, eng, split=False):
                start, size = GROUPS[gi]
                xm = xm_pool.tile([P, 2, size], bf16, tag="xm",
                                  name=f"xm_{gi}")
                if split:
                    for half in range(2):
                        eng.dma_start(xm[:, half, :],
                                      bwdT[:, half, start:start + size])
                else:
                    eng.dma_start(xm[:], bwdT[:, :, start:start + size])
                xm_tiles[gi] = xm

            def load_gather(gi):
                start, size = GROUPS[gi]
                ns = size // P
                xh = xh_pool.tile([P, ns, LD], bf16, tag="xh", name=f"xh_{gi}")
                for s in range(ns):
                    t = start // P + s
                    nc.gpsimd.indirect_dma_start(
                        out=xh[:, s, :],
                        out_offset=None,
                        in_=fwd[:],
                        in_offset=IndirectOffsetOnAxis(ap=hT_sb[:, t:t + 1], axis=0),
                    )
                xg_tiles[gi] = xh

            # scalar: first modifier slab has top priority, then bias pack,
            # identity, and a dummy ACT that preloads the tanh table
            load_xm(0, nc.scalar, split=True)
            load_gather(0)

            bp_sb = const_pool.tile([P, 10], f32)
            nc.scalar.dma_start(bp_sb[:], bpack[:])

            ident = const_pool.tile([P, P], bf16)
            nc.scalar.dma_start(ident[:], identw[:])

            scratch_sb = const_pool.tile([P, 1], f32)
            nc.scalar.activation(
                out=scratch_sb[:], in_=warm_sb[:, 0:1], func=Tanh, bias=0.0,
            )

            # sync: w1 k-chunks, modifier halves (kc 2,3) first
            w1_sb = [const_pool.tile([P, 2 * HID], bf16, tag=f"w1_{kc}",
                                     name=f"w1_{kc}")
                     for kc in range(4)]
            for col in range(2):
                nc.sync.dma_start(
                    w1_sb[2][:, col * HID:(col + 1) * HID],
                    w1[2 * P:3 * P, col * HID:(col + 1) * HID])
            nc.sync.dma_start(w1_sb[3][:], w1[3 * P:4 * P, :])

            load_xm(1, nc.scalar)
            load_gather(1)

            for kc in (0, 1):
                nc.sync.dma_start(w1_sb[kc][:], w1[kc * P:(kc + 1) * P, :])

            load_gather(2)

            # sync: w2 in two jc halves
            w2_sb = const_pool.tile([P, 2, 8, HID2 // 2], bf16)
            for jc in range(2):
                nc.sync.dma_start(w2_sb[:, jc], w2p[:, jc])

            load_gather(3)

            # sync: remaining cold tensors
            w3_sb = const_pool.tile([P, 2, NREL], bf16)
            nc.sync.dma_start(w3_sb[:], w3p[:])

            for gi in range(4, NG):
                load_gather(gi)
            # xm slabs for groups 2..7 are issued from inside the pipeline
            # (scalar queue) so the ACT stream paces them — keeps the early
            # HBM window clear for w1/w2 and the gathers

            xT_tiles = [None] * NG
            h1_tiles = [[None] * 8 for _ in range(NG)]
            ph_half = {}

            def emit_transpose(gi):
                start, size = GROUPS[gi]
                xh = xg_tiles[gi]
                xTs = []
                for kc in range(2):  # head half only
                    col = kc * P
                    pt = pt_pool.tile([P, size], bf16, tag="pt",
                                      name=f"pt_{gi}_{kc}")
                    for s in range(size // P):
                        nc.tensor.transpose(
                            pt[:, s * P:(s + 1) * P],
                            xh[:, s, col:col + P], ident[:])
                    xT = xT_pool.tile([P, size], bf16, tag="xT",
                                      name=f"xT_{gi}_{kc}")
                    nc.vector.tensor_copy(out=xT[:], in_=pt[:])
                    xTs.append(xT)
                xT_tiles[gi] = xTs

            def emit_l1_a(gi, half):
                """Modifier pass: open 4 hc accumulations (kc 2,3)."""
                start, size = GROUPS[gi]
                xm = xm_tiles[gi]
                phs = []
                for hc in range(4 * half, 4 * half + 4):
                    ph = ph_pool.tile([P, size], f32, tag="ph",
                                      name=f"ph_{gi}_{hc}")
                    phs.append(ph)
                    for i, kc in enumerate((2, 3)):
                        nc.tensor.matmul(
                            out=ph[:],
                            lhsT=w1_sb[kc][:, hc * P:(hc + 1) * P],
                            rhs=xm[:, kc - 2, :],
                            start=(i == 0),
                            stop=False,
                        )
                ph_half[(gi, half)] = phs

            def emit_l1_b(gi, half):
                """Head pass: close the accumulations, then tanh."""
                start, size = GROUPS[gi]
                xTs = xT_tiles[gi]
                phs = ph_half[(gi, half)]
                for j, hc in enumerate(range(4 * half, 4 * half + 4)):
                    for i, kc in enumerate((0, 1)):
                        nc.tensor.matmul(
                            out=phs[j][:],
                            lhsT=w1_sb[kc][:, hc * P:(hc + 1) * P],
                            rhs=xTs[kc][:],
                            start=False,
                            stop=(i == 1),
                        )
                for j, hc in enumerate(range(4 * half, 4 * half + 4)):
                    h1 = h1_pool.tile([P, size], bf16, tag="h1",
                                      name=f"h1_{gi}_{hc}")
                    nc.scalar.activation(
                        out=h1[:], in_=phs[j][:], func=Tanh,
                        bias=bp_sb[:, hc:hc + 1],
                    )
                    h1_tiles[gi][hc] = h1

            h2_tiles = [[None, None] for _ in range(NG)]

            def emit_l2_jc(gi, jc):
                start, size = GROUPS[gi]
                h1s = h1_tiles[gi]
                pj = pj_pool.tile([P, size], f32, tag="pj",
                                  name=f"pj_{gi}_{jc}")
                for kc in range(8):
                    nc.tensor.matmul(
                        out=pj[:],
                        lhsT=w2_sb[:, jc, kc, :],
                        rhs=h1s[kc][:],
                        start=(kc == 0),
                        stop=(kc == 7),
                    )
                h2 = h2_pool.tile([P, size], bf16, tag="h2",
                                  name=f"h2_{gi}_{jc}")
                nc.scalar.activation(
                    out=h2[:], in_=pj[:], func=Tanh,
                    bias=bp_sb[:, 8 + jc:9 + jc],
                )
                h2_tiles[gi][jc] = h2

            def emit_l3(gi):
                start, size = GROUPS[gi]
                h2s = h2_tiles[gi]
                po = pt_pool.tile([NREL, size], f32, tag="pt",
                                  name=f"po_{gi}")
                for kc in range(2):
                    nc.tensor.matmul(
                        out=po[:],
                        lhsT=w3_sb[:, kc, :],
                        rhs=h2s[kc][:],
                        start=(kc == 0),
                        stop=(kc == 1),
                    )
                o = out_pool.tile([NREL, size], bf16, tag="o",
                                  name=f"o_{gi}")
                nc.vector.tensor_copy(out=o[:], in_=po[:])
                nc.sync.dma_start(outT[:, start:start + size], o[:])

            # --- pipeline: two-pass L1 (lo/hi); the previous group's L2/L3
            # pieces are spread through this group's passes so ACT latency
            # and PSUM-bank recycling never stall the PE ---
            for gi in range(NG):
                emit_l1_a(gi, 0)
                if gi == 0:
                    emit_warm(6)      # fill until gather(0) has landed
                if gi == 1:
                    emit_warm(2)
                if gi >= 1:
                    emit_l2_jc(gi - 1, 0)
                emit_transpose(gi)
                emit_l1_b(gi, 0)
                if gi + 2 < NG:
                    load_xm(gi + 2, nc.scalar)
                if gi >= 1:
                    emit_l2_jc(gi - 1, 1)
                emit_l1_a(gi, 1)
                emit_l1_b(gi, 1)
                if gi >= 1:
                    emit_l3(gi - 1)
            emit_l2_jc(NG - 1, 0)
            emit_l2_jc(NG - 1, 1)
            emit_l3(NG - 1)

    nc.finalize()
    return nc


def _prep_inputs(inputs, rhidLayerFOH, rhidLayerFOM, rcatBias, rhid2Layer,
                 rhid2Bias, routLayer, routBias, heads):
    wdt = ml_dtypes.bfloat16
    inputs = np.asarray(inputs, dtype=np.float32)
    heads = np.asarray(heads)

    fwd = np.ascontiguousarray(inputs[:, 0, :]).astype(wdt)
    bwd_full = inputs[:, 1, :]
    mods_pad = np.concatenate(
        [np.arange(1, N_TOKENS), [N_TOKENS - 1]]).astype(np.int64)
    heads_pad = np.concatenate([heads.astype(np.int64), [0]]).astype(np.int32)

    w1 = np.ascontiguousarray(
        np.concatenate([np.asarray(rhidLayerFOH), np.asarray(rhidLayerFOM)],
                       axis=1)).astype(wdt)                      # [512, 1024]
    w2p = np.ascontiguousarray(
        np.asarray(rhid2Layer, dtype=np.float32)
        .reshape(8, P, 2, HID2 // 2).transpose(1, 2, 0, 3)).astype(wdt)
    w3p = np.ascontiguousarray(
        np.asarray(routLayer, dtype=np.float32)
        .reshape(2, P, NREL).transpose(1, 0, 2)).astype(wdt)
    b1 = np.asarray(rcatBias, dtype=np.float32).reshape(8, P).T
    b2 = np.asarray(rhid2Bias, dtype=np.float32).reshape(2, P).T
    bpack = np.ascontiguousarray(np.concatenate([b1, b2], axis=1))

    in_maps = []
    for c in range(NCORES):
        sl = slice(c * EPC, (c + 1) * EPC)
        bwd_c = bwd_full[mods_pad[sl]]                           # [2048, 256]
        bwdT_c = np.ascontiguousarray(
            bwd_c.T.reshape(2, P, EPC).transpose(1, 0, 2)).astype(wdt)
        headsT_c = np.ascontiguousarray(heads_pad[sl].reshape(SUB, P).T)
        in_maps.append({
            "fwd": fwd, "bwdT": bwdT_c, "headsT": headsT_c,
            "w1": w1, "w2p": w2p, "w3p": w3p, "bpack": bpack,
            "identw": np.eye(P, dtype=wdt),
        })
    return in_maps


def kernel(inputs, rhidLayerFOH, rhidLayerFOM, rcatBias, rhid2Layer, rhid2Bias,
           routLayer, routBias, heads):
    global LAST_RESULTS

    in_maps = _prep_inputs(inputs, rhidLayerFOH, rhidLayerFOM, rcatBias,
                           rhid2Layer, rhid2Bias, routLayer, routBias, heads)

    if "nc" not in _CACHE:
        _CACHE["nc"] = _build()
    nc = _CACHE["nc"]

    trace_dir = os.environ.get("KERNEL_TRACE_DIR") or None
    res = run_bass_kernel_spmd(nc, in_maps, list(range(NCORES)), tmpdir=trace_dir)
    LAST_RESULTS = res

    outT = np.concatenate(
        [np.asarray(r["outT"], dtype=np.float32) for r in res.results], axis=1)
    out = outT.T[:E] + np.asarray(routBias, dtype=np.float32)[None, :]
    return np.ascontiguousarray(out).astype(np.float32)


# revision 29
# speedup vs baseline: 1.0256x; 1.0256x over previous
"""ConcatRelationModule Bass kernel for 8 trn2 NeuronCores — v12.

Per edge e in [0, 16383):
    x      = concat(inputs[heads[e], 0, :], inputs[e + 1, 1, :])     # [512]
    h      = tanh(concat(x @ W_FOH, x @ W_FOM) + b1)                 # [1024]
    h2     = tanh(h @ W2 + b2)                                       # [256]
    out[e] = h2 @ W3 + b3                                            # [E, 64]

v12 core idea: the gather (gpsimd software descriptor generation,
~1.1us per 128 rows) is the pacing item early on, and any PE idle
>~3.4us re-throttles the HAM clock gate to half rate.  So L1 is split
into two passes per group of 256 edges:
  pass A: the modifier half (kc 2,3) — fed by a host-pretransposed
          direct DMA, no gather dependency
  pass B: the head half (kc 0,1) — closes the accumulation once the
          gather+transpose has landed
Eight hc accumulations stay open across the passes by packing hc pairs
into single PSUM banks ([P, 2, 256] f32 = one 2KB bank), so a group
needs 4 banks.  L2/L3 lag L1 by one group.  DMA deliveries are
priority-ordered: hT+gathers on gpsimd only; xm/bp/ident early on
scalar; w1 then w2 then cold tensors on sync.  Warm-up matmuls gated
only on a small DVE memset keep the PE (and HAM) busy from ~7us.
Output is stored bf16 and converted (+bias) on host.
"""

import os

import numpy as np
import ml_dtypes

import concourse.bass as bass
import concourse.bacc as bacc
import concourse.mybir as mybir
import concourse.tile as tile
from concourse.bass import IndirectOffsetOnAxis
from concourse.bass_utils import run_bass_kernel_spmd

N_TOKENS = 16384
LD = 256
HID = 512
HID2 = 256
NREL = 64
NCORES = 8
E = N_TOKENS - 1
EPC = N_TOKENS // NCORES  # 2048
P = 128
SUB = EPC // P            # 16
N_WARMUP = 12

# variable group sizes: small first groups (fast pipeline ramp: less
# data needed before the first matmuls / first gather) and small last
# groups (short drain tail)
GSIZES = [128, 128, 512, 512, 512, 128, 128]
GROUPS = []
_o = 0
for _s in GSIZES:
    GROUPS.append((_o, _s))
    _o += _s
assert _o == EPC
NG = len(GROUPS)
GMAX = max(GSIZES)

LAST_RESULTS = None
_CACHE = {}


def _build():
    bf16 = mybir.dt.bfloat16
    f32 = mybir.dt.float32

    nc = bacc.Bacc()
    fwd = nc.declare_dram_parameter("fwd", [N_TOKENS, LD], bf16, isOutput=False)
    bwdT = nc.declare_dram_parameter("bwdT", [P, 2, EPC], bf16, isOutput=False)
    headsT = nc.declare_dram_parameter(
        "headsT", [P, SUB], mybir.dt.int32, isOutput=False)
    w1 = nc.declare_dram_parameter("w1", [2 * LD, 2 * HID], bf16, isOutput=False)
    w2p = nc.declare_dram_parameter("w2p", [P, 2, 8, HID2 // 2], bf16,
                                    isOutput=False)
    w3p = nc.declare_dram_parameter("w3p", [P, 2, NREL], bf16, isOutput=False)
    bpack = nc.declare_dram_parameter("bpack", [P, 10], f32, isOutput=False)
    identw = nc.declare_dram_parameter("identw", [P, P], bf16, isOutput=False)
    outT = nc.declare_dram_parameter("outT", [NREL, EPC], bf16, isOutput=True)

    Tanh = mybir.ActivationFunctionType.Tanh

    with tile.TileContext(nc) as tc:
        with (
            tc.tile_pool(name="const", bufs=1) as const_pool,
            tc.tile_pool(name="xh", bufs=NG) as xh_pool,
            tc.tile_pool(name="xm", bufs=NG) as xm_pool,
            tc.tile_pool(name="xT", bufs=4) as xT_pool,
            tc.tile_pool(name="h1", bufs=16) as h1_pool,
            tc.tile_pool(name="h2", bufs=4) as h2_pool,
            tc.tile_pool(name="outs", bufs=3) as out_pool,
            # PSUM: ph 4 banks (one open hc accumulation each), pt 2, pj 2.
            # NOTE: start_tensor_calc=True clears has_written for the WHOLE
            # bank, so only one open accumulation region may live per bank.
            tc.tile_pool(name="pt", bufs=2, space="PSUM") as pt_pool,
            tc.tile_pool(name="ph", bufs=4, space="PSUM") as ph_pool,
            tc.tile_pool(name="pj", bufs=2, space="PSUM") as pj_pool,
        ):
            # --- prologue (per-engine emission order == queue order) ---
            # DVE: warm-up scratch; first 128 cols land fast so the PE
            # warm-ups can start ~0.6us earlier
            warm_sb = const_pool.tile([P, 512], bf16)
            nc.vector.memset(warm_sb[:, 0:P], 0)
            nc.vector.memset(warm_sb[:, P:512], 0)

            # gpsimd: headsT (first two columns split out: gather(0) and
            # gather(1) gate only on the tiny first transfer), then gathers
            hT_sb = const_pool.tile([P, SUB], mybir.dt.int32)
            nc.gpsimd.dma_start(hT_sb[:, 0:2], headsT[:, 0:2])
            nc.gpsimd.dma_start(hT_sb[:, 2:SUB], headsT[:, 2:SUB])

            # PE warm-ups keep the HAM clock gate at K=8/8 (any >3.4us PE
            # idle re-throttles to half rate).  A first block runs during
            # the DMA prologue; more are sprinkled between the early
            # pipeline stages as dependency-free gap fillers.
            wps = pt_pool.tile([P, 512], f32, tag="pt", name="warmup")

            def emit_warm(n, free=512):
                for _ in range(n):
                    nc.tensor.matmul(
                        out=wps[:, 0:free], lhsT=warm_sb[:, 0:P],
                        rhs=warm_sb[:, 0:free], start=True, stop=True,
                    )

            emit_warm(4, free=P)
            emit_warm(7)

            xg_tiles = [None] * NG
            xm_tiles = [None] * NG

            def load_xm(gi, eng, split=False):
                start, size = GROUPS[gi]
                xm = xm_pool.tile([P, 2, size], bf16, tag="xm",
                                  name=f"xm_{gi}")
                if split:
                    for half in range(2):
                        eng.dma_start(xm[:, half, :],
                                      bwdT[:, half, start:start + size])
                else:
                    eng.dma_start(xm[:], bwdT[:, :, start:start + size])
                xm_tiles[gi] = xm

            def load_gather(gi):
                start, size = GROUPS[gi]
                ns = size // P
                xh = xh_pool.tile([P, ns, LD], bf16, tag="xh", name=f"xh_{gi}")
                for s in range(ns):
                    t = start // P + s
                    nc.gpsimd.indirect_dma_start(
                        out=xh[:, s, :],
                        out_offset=None,
                        in_=fwd[:],
                        in_offset=IndirectOffsetOnAxis(ap=hT_sb[:, t:t + 1], axis=0),
                    )
                xg_tiles[gi] = xh

            # scalar: first modifier slab has top priority, then bias pack,
            # identity, and a dummy ACT that preloads the tanh table
            load_xm(0, nc.scalar, split=True)
            load_gather(0)

            bp_sb = const_pool.tile([P, 10], f32)
            nc.scalar.dma_start(bp_sb[:], bpack[:])

            ident = const_pool.tile([P, P], bf16)
            nc.scalar.dma_start(ident[:], identw[:])

            scratch_sb = const_pool.tile([P, 1], f32)
            nc.scalar.activation(
                out=scratch_sb[:], in_=warm_sb[:, 0:1], func=Tanh, bias=0.0,
            )

            # sync: w1 k-chunks, modifier halves (kc 2,3) first
            w1_sb = [const_pool.tile([P, 2 * HID], bf16, tag=f"w1_{kc}",
                                     name=f"w1_{kc}")
                     for kc in range(4)]
            for col in range(2):
                nc.sync.dma_start(
                    w1_sb[2][:, col * HID:(col + 1) * HID],
                    w1[2 * P:3 * P, col * HID:(col + 1) * HID])
            nc.sync.dma_start(w1_sb[3][:], w1[3 * P:4 * P, :])

            load_xm(1, nc.scalar)
            load_gather(1)
            load_xm(2, nc.scalar)

            for kc in (0, 1):
                nc.sync.dma_start(w1_sb[kc][:], w1[kc * P:(kc + 1) * P, :])

            load_gather(2)

            # sync: w2 in two jc halves
            w2_sb = const_pool.tile([P, 2, 8, HID2 // 2], bf16)
            for jc in range(2):
                nc.sync.dma_start(w2_sb[:, jc], w2p[:, jc])

            load_gather(3)

            # sync: remaining cold tensors
            w3_sb = const_pool.tile([P, 2, NREL], bf16)
            nc.sync.dma_start(w3_sb[:], w3p[:])

            for gi in range(4, NG):
                load_gather(gi)
            # xm slabs for groups 2..7 are issued from inside the pipeline
            # (scalar queue) so the ACT stream paces them — keeps the early
            # HBM window clear for w1/w2 and the gathers

            xT_tiles = [None] * NG
            h1_tiles = [[None] * 8 for _ in range(NG)]
            ph_half = {}

            def emit_transpose(gi):
                start, size = GROUPS[gi]
                xh = xg_tiles[gi]
                xTs = []
                for kc in range(2):  # head half only
                    col = kc * P
                    pt = pt_pool.tile([P, size], bf16, tag="pt",
                                      name=f"pt_{gi}_{kc}")
                    for s in range(size // P):
                        nc.tensor.transpose(
                            pt[:, s * P:(s + 1) * P],
                            xh[:, s, col:col + P], ident[:])
                    xT = xT_pool.tile([P, size], bf16, tag="xT",
                                      name=f"xT_{gi}_{kc}")
                    nc.vector.tensor_copy(out=xT[:], in_=pt[:])
                    xTs.append(xT)
                xT_tiles[gi] = xTs

            def emit_l1_a(gi, half):
                """Modifier pass: open 4 hc accumulations (kc 2,3)."""
                start, size = GROUPS[gi]
                xm = xm_tiles[gi]
                phs = []
                for hc in range(4 * half, 4 * half + 4):
                    ph = ph_pool.tile([P, size], f32, tag="ph",
                                      name=f"ph_{gi}_{hc}")
                    phs.append(ph)
                    for i, kc in enumerate((2, 3)):
                        nc.tensor.matmul(
                            out=ph[:],
                            lhsT=w1_sb[kc][:, hc * P:(hc + 1) * P],
                            rhs=xm[:, kc - 2, :],
                            start=(i == 0),
                            stop=False,
                        )
                ph_half[(gi, half)] = phs

            def emit_l1_b(gi, half):
                """Head pass: close the accumulations, then tanh."""
                start, size = GROUPS[gi]
                xTs = xT_tiles[gi]
                phs = ph_half[(gi, half)]
                for j, hc in enumerate(range(4 * half, 4 * half + 4)):
                    for i, kc in enumerate((0, 1)):
                        nc.tensor.matmul(
                            out=phs[j][:],
                            lhsT=w1_sb[kc][:, hc * P:(hc + 1) * P],
                            rhs=xTs[kc][:],
                            start=False,
                            stop=(i == 1),
                        )
                for j, hc in enumerate(range(4 * half, 4 * half + 4)):
                    h1 = h1_pool.tile([P, size], bf16, tag="h1",
                                      name=f"h1_{gi}_{hc}")
                    nc.scalar.activation(
                        out=h1[:], in_=phs[j][:], func=Tanh,
                        bias=bp_sb[:, hc:hc + 1],
                    )
                    h1_tiles[gi][hc] = h1

            h2_tiles = [[None, None] for _ in range(NG)]

            def emit_l2_jc(gi, jc):
                start, size = GROUPS[gi]
                h1s = h1_tiles[gi]
                pj = pj_pool.tile([P, size], f32, tag="pj",
                                  name=f"pj_{gi}_{jc}")
                for kc in range(8):
                    nc.tensor.matmul(
                        out=pj[:],
                        lhsT=w2_sb[:, jc, kc, :],
                        rhs=h1s[kc][:],
                        start=(kc == 0),
                        stop=(kc == 7),
                    )
                h2 = h2_pool.tile([P, size], bf16, tag="h2",
                                  name=f"h2_{gi}_{jc}")
                nc.scalar.activation(
                    out=h2[:], in_=pj[:], func=Tanh,
                    bias=bp_sb[:, 8 + jc:9 + jc],
                )
                h2_tiles[gi][jc] = h2

            def emit_l3(gi):
                start, size = GROUPS[gi]
                h2s = h2_tiles[gi]
                po = pt_pool.tile([NREL, size], f32, tag="pt",
                                  name=f"po_{gi}")
                for kc in range(2):
                    nc.tensor.matmul(
                        out=po[:],
                        lhsT=w3_sb[:, kc, :],
                        rhs=h2s[kc][:],
                        start=(kc == 0),
                        stop=(kc == 1),
                    )
                o = out_pool.tile([NREL, size], bf16, tag="o",
                                  name=f"o_{gi}")
                nc.vector.tensor_copy(out=o[:], in_=po[:])
                nc.sync.dma_start(outT[:, start:start + size], o[:])

            # --- pipeline: two-pass L1 (lo/hi); the previous group's L2/L3
            # pieces are spread through this group's passes so ACT latency
            # and PSUM-bank recycling never stall the PE ---
            for gi in range(NG):
                emit_l1_a(gi, 0)
                if gi == 0:
                    emit_warm(4)      # fill until gather(0) has landed
                if gi >= 1:
                    emit_l2_jc(gi - 1, 0)
                emit_transpose(gi)
                emit_l1_b(gi, 0)
                if 1 <= gi and gi + 2 < NG:
                    load_xm(gi + 2, nc.scalar)
                if gi >= 1:
                    emit_l2_jc(gi - 1, 1)
                emit_l1_a(gi, 1)
                emit_l1_b(gi, 1)
                if gi >= 1:
                    emit_l3(gi - 1)
            emit_l2_jc(NG - 1, 0)
            emit_l2_jc(NG - 1, 1)
            emit_l3(NG - 1)

    nc.finalize()
    return nc


def _prep_inputs(inputs, rhidLayerFOH, rhidLayerFOM, rcatBias, rhid2Layer,
                 rhid2Bias, routLayer, routBias, heads):
    wdt = ml_dtypes.bfloat16
    inputs = np.asarray(inputs, dtype=np.float32)
    heads = np.asarray(heads)

    fwd = np.ascontiguousarray(inputs[:, 0, :]).astype(wdt)
    bwd_full = inputs[:, 1, :]
    mods_pad = np.concatenate(
        [np.arange(1, N_TOKENS), [N_TOKENS - 1]]).astype(np.int64)
    heads_pad = np.concatenate([heads.astype(np.int64), [0]]).astype(np.int32)

    w1 = np.ascontiguousarray(
        np.concatenate([np.asarray(rhidLayerFOH), np.asarray(rhidLayerFOM)],
                       axis=1)).astype(wdt)                      # [512, 1024]
    w2p = np.ascontiguousarray(
        np.asarray(rhid2Layer, dtype=np.float32)
        .reshape(8, P, 2, HID2 // 2).transpose(1, 2, 0, 3)).astype(wdt)
    w3p = np.ascontiguousarray(
        np.asarray(routLayer, dtype=np.float32)
        .reshape(2, P, NREL).transpose(1, 0, 2)).astype(wdt)
    b1 = np.asarray(rcatBias, dtype=np.float32).reshape(8, P).T
    b2 = np.asarray(rhid2Bias, dtype=np.float32).reshape(2, P).T
    bpack = np.ascontiguousarray(np.concatenate([b1, b2], axis=1))

    in_maps = []
    for c in range(NCORES):
        sl = slice(c * EPC, (c + 1) * EPC)
        bwd_c = bwd_full[mods_pad[sl]]                           # [2048, 256]
        bwdT_c = np.ascontiguousarray(
            bwd_c.T.reshape(2, P, EPC).transpose(1, 0, 2)).astype(wdt)
        headsT_c = np.ascontiguousarray(heads_pad[sl].reshape(SUB, P).T)
        in_maps.append({
            "fwd": fwd, "bwdT": bwdT_c, "headsT": headsT_c,
            "w1": w1, "w2p": w2p, "w3p": w3p, "bpack": bpack,
            "identw": np.eye(P, dtype=wdt),
        })
    return in_maps


def kernel(inputs, rhidLayerFOH, rhidLayerFOM, rcatBias, rhid2Layer, rhid2Bias,
           routLayer, routBias, heads):
    global LAST_RESULTS

    in_maps = _prep_inputs(inputs, rhidLayerFOH, rhidLayerFOM, rcatBias,
                           rhid2Layer, rhid2Bias, routLayer, routBias, heads)

    if "nc" not in _CACHE:
        _CACHE["nc"] = _build()
    nc = _CACHE["nc"]

    trace_dir = os.environ.get("KERNEL_TRACE_DIR") or None
    res = run_bass_kernel_spmd(nc, in_maps, list(range(NCORES)), tmpdir=trace_dir)
    LAST_RESULTS = res

    outT = np.concatenate(
        [np.asarray(r["outT"], dtype=np.float32) for r in res.results], axis=1)
    out = outT.T[:E] + np.asarray(routBias, dtype=np.float32)[None, :]
    return np.ascontiguousarray(out).astype(np.float32)
